# revision 1
# baseline (speedup 1.0000x reference)
"""Trainium2 Bass kernel for causal self-attention (B=4, S=2048, C=2048, H=16).

Sharding over 8 NeuronCores: core = 2*batch + head_group
  - data-parallel over the 4 batches (outer axis)
  - tensor-parallel over heads within a batch: 2 groups x 8 heads
Each core computes qkv projection for its head group, block-causal
flash-style attention for its 8 heads, and a partial output projection
(contraction over its 1024 w_proj rows). The host sums the two partial
outputs per batch and adds b_proj ("all-reduce" done during unshard).

Device compute is bf16 with f32 PSUM accumulation.
"""

from contextlib import ExitStack

import numpy as np
import ml_dtypes

import concourse.bass as bass
import concourse.tile as tile
from concourse import bacc, mybir
from concourse.bass_utils import run_bass_kernel_spmd

BF16 = mybir.dt.bfloat16
F32 = mybir.dt.float32
ExpF = mybir.ActivationFunctionType.Exp
NPBF16 = ml_dtypes.bfloat16

B, S, C, H = 4, 2048, 2048, 16
D = 128
N_CORES = 8
NH = 8              # heads per core
NQ = NH * D         # 1024 q (=k=v) columns per core
SQT = 512           # sq tile width


def _build(compile=True, reps=1):
    CK = C // 128            # contraction chunks
    NST = S // SQT           # s tiles of 512
    NSC = S // 128           # s chunks of 128
    NB_QK = 2 * NQ // 128    # q+k output chunks of 128
    NVT = NQ // 512          # v n-tiles
    ET = C // 512            # proj e tiles
    scale = 1.0 / float(np.sqrt(float(D)))

    nc = bacc.Bacc(
        "TRN2",
        target_bir_lowering=False,
        debug=False,
        enable_asserts=False,
        num_devices=N_CORES,
    )
    xT_d = nc.dram_tensor("xT", [128, (S // 128) * (C // 128) * 128], BF16, kind="ExternalInput").ap()
    wqkv_d = nc.dram_tensor("wqkv", [C, 3 * NQ], BF16, kind="ExternalInput").ap()
    bqkv_d = nc.dram_tensor("bqkv", [1, 3 * NQ], BF16, kind="ExternalInput").ap()
    bqkvcol_d = nc.dram_tensor(
        "bqkvcol", [128, 2 * NQ // 128], BF16, kind="ExternalInput"
    ).ap()
    mtri_d = nc.dram_tensor("mtri", [128, 256], BF16, kind="ExternalInput").ap()
    wproj_d = nc.dram_tensor("wproj", [NQ, C], BF16, kind="ExternalInput").ap()
    out_d = nc.dram_tensor("out", [S, C], F32, kind="ExternalOutput").ap()

    with tile.TileContext(nc) as tc, ExitStack() as top:
        persist = top.enter_context(tc.tile_pool(name="persist", bufs=1))
        # q_sb/k_sb: [d, h, s]; after attention, yT_h overwrites q_sb[:, h, :]
        q_sb = persist.tile([128, NH, S], BF16, tag="q")
        k_sb = persist.tile([128, NH, S], BF16, tag="k")
        # v_sb: [s%128, s//128, h*128+d], natural v layout per s-chunk
        v_sb = persist.tile([128, NSC, NQ], BF16, tag="v")
        mtri_sb = persist.tile([128, 256], BF16, tag="mtri")
        bias_sb = persist.tile([1, 3 * NQ], BF16, tag="bias")
        # q/k bias as per-partition columns: bias_col[p, nb] = bqkv[nb*128 + p]
        bias_col = persist.tile([128, 2 * NQ // 128], BF16, tag="bias_col")
        ones_bf = persist.tile([1, 512], BF16, tag="ones_bf")
        ones_col_bf = persist.tile([128, 1], BF16, tag="ones_col_bf")
        ones_col_f = persist.tile([128, 1], F32, tag="ones_col_f")
        ones_row_f = persist.tile([1, 128], F32, tag="ones_row_f")

        nc.sync.dma_start(out=mtri_sb, in_=mtri_d)
        nc.sync.dma_start(out=bias_sb, in_=bqkv_d)
        nc.sync.dma_start(out=bias_col, in_=bqkvcol_d)
        nc.vector.memset(ones_bf, 1.0)
        nc.vector.memset(ones_col_bf, 1.0)
        nc.vector.memset(ones_col_f, 1.0)
        nc.vector.memset(ones_row_f, 1.0)
        ltri = mtri_sb[:, 0:128]      # strict lower-tri (sk>sq) 0/1
        negdiag = mtri_sb[:, 128:256]  # diag(-1e9)

        for _rep in range(reps):
            # ---------------- Phase 1: QKV projection ----------------
            # x fully resident; each weight column chunk read exactly once.
            # Section order: v, then k, then q — so attention t=0 unblocks asap.
            with (
                tc.tile_pool(name="ph1x", bufs=1) as ph1x,
                tc.tile_pool(name="ph1wv", bufs=2) as ph1wv,
                tc.tile_pool(name="ph1wqk", bufs=3) as ph1wqk,
                tc.tile_pool(name="ps1", bufs=4, space="PSUM") as ps1,
            ):
                def load_wv(nt):
                    wt = ph1wv.tile([128, CK, 256], BF16, tag="wv")
                    nc.sync.dma_start(
                        out=wt,
                        in_=wqkv_d[:, 2 * NQ + nt * 256 : 2 * NQ + (nt + 1) * 256].rearrange(
                            "(ck p) n -> p ck n", p=128
                        ),
                    )
                    return wt

                wt0 = load_wv(0)  # ahead of the x stream in the DMA queue
                xfull = ph1x.tile([128, CK, S], BF16, tag="xf")
                dma_engs = [nc.sync, nc.scalar]
                for sc in range(NSC):
                    # slab sc: per-partition contiguous 4KB run from swizzled xT;
                    # rotate DMA queues so the x stream isn't single-queue-bound
                    dma_engs[sc % 2].dma_start(
                        out=xfull[:, :, bass.ts(sc, 128)],
                        in_=xT_d[:, sc * CK * 128 : (sc + 1) * CK * 128].rearrange(
                            "p (ck sl) -> p ck sl", ck=CK
                        ),
                    )
                def emit_qk(sec, hh):
                    nb = sec * NH + hh
                    wt = ph1wqk.tile([128, CK, 128], BF16, tag="wqk", name="wqk")
                    nc.sync.dma_start(
                        out=wt,
                        in_=wqkv_d[:, bass.ts(nb, 128)].rearrange(
                            "(ck p) n -> p ck n", p=128
                        ),
                    )
                    dest = q_sb if sec == 0 else k_sb
                    for st in range(NST):
                        ps = ps1.tile([128, 512], F32, tag="psqk", bufs=4, name="psqk")
                        for ck in range(CK):
                            nc.tensor.matmul(
                                ps,
                                lhsT=wt[:, ck, :],
                                rhs=xfull[:, ck, bass.ts(st, 512)],
                                start=(ck == 0),
                                stop=(ck == CK - 1),
                            )
                        # copy + per-partition bias add on ScalarE
                        nc.scalar.add(
                            dest[:, hh, bass.ts(st, 512)], ps, bias_col[:, nb : nb + 1]
                        )

                # v: n-tiles of 256, psum[s 128, n 256]
                for nt in range(NQ // 256):
                    wt = wt0 if nt == 0 else load_wv(nt)
                    for sc in range(NSC):
                        ps = ps1.tile([128, 512], F32, tag="psv", bufs=4)
                        psv = ps[:, :256]
                        for ck in range(CK):
                            nc.tensor.matmul(
                                psv,
                                lhsT=xfull[:, ck, bass.ts(sc, 128)],
                                rhs=wt[:, ck, :],
                                start=(ck == 0),
                                stop=False,
                            )
                        # bias: out[s, n] += 1 * b[n]
                        nc.tensor.matmul(
                            psv,
                            lhsT=ones_bf[:, :128],
                            rhs=bias_sb[:, 2 * NQ + nt * 256 : 2 * NQ + (nt + 1) * 256],
                            start=False,
                            stop=True,
                        )
                        nc.vector.tensor_copy(v_sb[:, sc, bass.ts(nt, 256)], psv)
                # k then q, transposed: psum[n 128, s 512]
                for hh in range(NH):
                    emit_qk(1, hh)
                for hh in range(NH):
                    emit_qk(0, hh)

            # -------- Phase 2+3: block-causal attention + projection --------
            with (
                tc.tile_pool(name="att", bufs=4) as att,
                tc.tile_pool(name="ph3", bufs=2) as ph3,
                tc.tile_pool(name="ps2", bufs=1, space="PSUM") as ps2,
            ):
                wp = ph3.tile([128, NH, C], BF16, tag="wp", bufs=1)
                nc.sync.dma_start(out=wp, in_=wproj_d.rearrange("(h p) e -> p h e", p=128))

                def emit_proj(t_src, lo, hi, tag="po", bufs=1):
                    tiles = [
                        (sqc, et)
                        for sqc in range(4 * t_src, 4 * (t_src + 1))
                        for et in range(ET)
                    ]
                    for sqc, et in tiles[lo:hi]:
                        ps_o = ps2.tile([128, 512], F32, tag=tag, bufs=bufs)
                        for hp in range(NH):
                            nc.tensor.matmul(
                                ps_o,
                                lhsT=q_sb[:, hp, bass.ts(sqc, 128)],
                                rhs=wp[:, hp, bass.ts(et, 512)],
                                start=(hp == 0),
                                stop=(hp == NH - 1),
                            )
                        o_sb = ph3.tile([128, 512], F32, tag="o")
                        nc.vector.tensor_copy(o_sb, ps_o)
                        nc.sync.dma_start(
                            out=out_d[bass.ts(sqc, 128), bass.ts(et, 512)], in_=o_sb
                        )

                for t in range(NST):
                    tsl = bass.ts(t, SQT)
                    nsk = 4 * t + 4  # block-causal sk chunks
                    pending = None   # previous head awaiting normalization

                    def flush_pending():
                        nonlocal pending
                        if pending is None:
                            return
                        yu_p, rs_p, h_p = pending
                        ps_bc = ps2.tile([128, 512], F32, tag="bc", bufs=1)
                        nc.tensor.matmul(
                            ps_bc, lhsT=ones_row_f, rhs=rs_p, start=True, stop=True
                        )
                        bc_sb = att.tile([128, 512], F32, tag="bcs", bufs=2)
                        nc.vector.tensor_copy(bc_sb, ps_bc)
                        # yT (bf16) overwrites q_sb[:, h_p, tsl]
                        nc.vector.tensor_mul(q_sb[:, h_p, tsl], yu_p, bc_sb)
                        pending = None

                    for h in range(NH):
                        ps_yu = ps2.tile([128, 512], F32, tag="yu", bufs=2)
                        ps_rs = ps2.tile([1, 512], F32, tag="rs", bufs=1)
                        # row-sum partials on DVE; two interleaved chains for long
                        # blocks so the serial adds stay shorter than PE's work
                        acc = att.tile([128, 512], F32, tag="acc", bufs=2, name="acc")
                        acc2 = None
                        if nsk > 8:
                            acc2 = att.tile([128, 512], F32, tag="acc2", bufs=2, name="acc2")
                        sc_tiles = {}

                        def emit_scores(j, h=h):
                            off = 0 if j < 4 * t else (j - 4 * t) * 128
                            w = 512 - off
                            diag = j >= 4 * t
                            ps_sc = ps2.tile([128, 512], F32, tag="sc", bufs=3)
                            # scoresT[sk, sq] = k_h.T q_h (live sq columns only)
                            nc.tensor.matmul(
                                ps_sc[:, :w],
                                lhsT=k_sb[:, h, bass.ts(j, 128)],
                                rhs=q_sb[:, h, t * SQT + off : (t + 1) * SQT],
                                start=True,
                                stop=not diag,
                            )
                            if diag:
                                # causal mask on PE: scoresT[p, f] += -1e9 * (p > f)
                                nc.tensor.matmul(
                                    ps_sc[:, :128],
                                    lhsT=negdiag,
                                    rhs=ltri,
                                    start=False,
                                    stop=True,
                                )
                            sc_tiles[j] = (ps_sc, off, w)

                        emit_scores(0)
                        if nsk > 1:
                            emit_scores(1)
                        for j in range(nsk):
                            ps_sc, off, w = sc_tiles.pop(j)
                            e = att.tile([128, 512], BF16, tag="e", bufs=10)
                            nc.scalar.activation(
                                out=e[:, off:], in_=ps_sc[:, :w], func=ExpF, scale=scale
                            )
                            if j + 2 < nsk:
                                emit_scores(j + 2)
                            if j == 0:
                                flush_pending()
                            # row sums (live region; first touch is full width)
                            tgt = acc if (acc2 is None or j % 2 == 0) else acc2
                            if j <= (0 if acc2 is None else 1):
                                nc.vector.tensor_copy(tgt, e)
                            else:
                                nc.vector.tensor_add(
                                    tgt[:, off:], tgt[:, off:], e[:, off:]
                                )
                            # yu[d, sq] += v[sk, d].T @ e[sk, sq] (live region)
                            nc.tensor.matmul(
                                ps_yu[:, off:],
                                lhsT=v_sb[:, j, bass.ts(h, 128)],
                                rhs=e[:, off:],
                                start=(j == 0),
                                stop=(j == nsk - 1),
                            )
                        # partition-reduce the accumulated exp sums on PE
                        nc.tensor.matmul(
                            ps_rs,
                            lhsT=ones_col_f,
                            rhs=acc,
                            start=True,
                            stop=(acc2 is None),
                        )
                        if acc2 is not None:
                            nc.tensor.matmul(
                                ps_rs, lhsT=ones_col_f, rhs=acc2, start=False, stop=True
                            )
                        rs_sb = att.tile([1, 512], F32, tag="rsb", bufs=2)
                        nc.vector.reciprocal(rs_sb, ps_rs)
                        # interleave prev t-block's projection tiles: fills PE
                        # while this head's reciprocal completes on DVE
                        if t > 0:
                            emit_proj(t - 1, 2 * h, 2 * h + 2)
                        pending = (ps_yu, rs_sb, h)
                    flush_pending()
                    if t == NST - 1:
                        emit_proj(t, 0, 4 * ET, tag="yu", bufs=2)

    if compile:
        nc.compile()
    return nc


def _make_mtri():
    """[:, :128] strict lower-tri (sk>sq -> 1); [:, 128:] diag(-1e9)."""
    m = np.zeros((128, 256), np.float32)
    m[:, :128] = np.tril(np.ones((128, 128), np.float32), -1)
    m[:, 128:] = np.diag(np.full(128, -1e9, np.float32))
    return m.astype(NPBF16)


_NC_CACHE = None


def _get_nc():
    global _NC_CACHE
    if _NC_CACHE is None:
        _NC_CACHE = _build()
    return _NC_CACHE


def _make_in_maps(x, w_qkv, b_qkv, w_proj):
    mtri = _make_mtri()
    in_maps = []
    for core in range(N_CORES):
        b = core // 2
        g = core % 2
        cs = slice(g * NQ, (g + 1) * NQ)
        xb = np.asarray(x[b], np.float32).astype(NPBF16)
        # xh[p, sc, ck, sl] = x[sc*128+sl, ck*128+p], flattened to [128, S*C/128]
        xT = np.ascontiguousarray(
            xb.reshape(S // 128, 128, C // 128, 128).transpose(3, 0, 2, 1)
        ).reshape(128, (S // 128) * (C // 128) * 128)
        wqkv_c = np.ascontiguousarray(
            np.concatenate(
                [w_qkv[:, cs], w_qkv[:, C:][:, cs], w_qkv[:, 2 * C:][:, cs]], axis=1
            )
        ).astype(NPBF16)
        bqkv_c = np.concatenate(
            [b_qkv[cs], b_qkv[C:][cs], b_qkv[2 * C:][cs]]
        )[None, :].astype(NPBF16)
        bqkvcol = np.ascontiguousarray(
            bqkv_c[0, : 2 * NQ].reshape(2 * NQ // 128, 128).T
        )
        wp = np.ascontiguousarray(w_proj[cs, :]).astype(NPBF16)
        in_maps.append(
            {
                "xT": xT,
                "wqkv": wqkv_c,
                "bqkv": bqkv_c,
                "bqkvcol": bqkvcol,
                "mtri": mtri,
                "wproj": wp,
            }
        )
    return in_maps


def kernel(x, w_qkv, b_qkv, w_proj, b_proj):
    x = np.asarray(x, np.float32)
    w_qkv = np.asarray(w_qkv, np.float32)
    b_qkv = np.asarray(b_qkv, np.float32)
    w_proj = np.asarray(w_proj, np.float32)
    b_proj = np.asarray(b_proj, np.float32)

    nc = _get_nc()
    in_maps = _make_in_maps(x, w_qkv, b_qkv, w_proj)
    res = run_bass_kernel_spmd(nc, in_maps, core_ids=list(range(N_CORES)))

    out = np.empty((B, S, C), np.float32)
    for b in range(B):
        out[b] = res.results[2 * b]["out"] + res.results[2 * b + 1]["out"]
        out[b] += b_proj[None, :]
    return out

